# revision 25
# baseline (speedup 1.0000x reference)
"""Trainium2 Bass kernel for nn_MultiHeadAttention (B=4,T=1024,C=1024,H=16).

Sharding: 8 cores = 4 batches x 2 query-halves. Each core computes, for its
batch b and its 512 query rows:
  V projection (natural layout, mask folded in, +mask column for denominator),
  then per head-pair: Q^T/K^T projection chunks, S^T = K^T.T @ Q^T (row-tiled
  head pairs, D=64 contraction), E^T = exp(0.125*S^T) on ACT (unmasked),
  O^T+denominator via one augmented matmul lhsT=[V_h*m | m], normalize via
  broadcast reciprocal; finally Y = O^T.T @ Wo + (bv@Wo+bo) and LayerNorm.
Host gathers the 8 [512,1024] outputs into [4,1024,1024].
"""

import os
import sys

import numpy as np

for _p in ("/opt/trn_rl_repo", "/root/.axon_site/_ro/trn_rl_repo"):
    if os.path.isdir(_p) and _p not in sys.path:
        sys.path.append(_p)

import ml_dtypes  # noqa: E402
import concourse.bass as bass  # noqa: E402
import concourse.mybir as mybir  # noqa: E402
import concourse.tile as tile  # noqa: E402
from concourse import bacc  # noqa: E402
from concourse.bass_utils import run_bass_kernel_spmd  # noqa: E402

BF16 = mybir.dt.bfloat16
F32 = mybir.dt.float32
NPBF16 = ml_dtypes.bfloat16

B, T, C, H = 4, 1024, 1024, 16
D = C // H          # 64
P = 128             # partitions
NC = C // P         # 8 chunks of C
NT = T // P         # 8 chunks of T
TQ = T // 2         # 512 query rows per core
NQ = TQ // P        # 4 query chunks
NPAIR = H // 2      # 8 head pairs
EPS = 1e-5

_CACHE = {}
LAST_RESULTS = None


def _ensure_ntff_hook():
    """Register the axon NTFF profiling hook if the image's antenv lacks it."""
    try:
        import antenv.axon_hooks  # noqa: F401
        return
    except ImportError:
        pass
    try:
        import types

        import antenv
        from trn_agent_boot.trn_boot import _ntff_profile_via_ctypes

        mod = types.ModuleType("antenv.axon_hooks")
        state = {"hook": None}
        mod.set_axon_ntff_profile_hook = lambda h: state.__setitem__("hook", h)
        mod.get_axon_ntff_profile_hook = lambda: state["hook"]
        sys.modules["antenv.axon_hooks"] = mod
        antenv.axon_hooks = mod
        hook = _ntff_profile_via_ctypes("/opt/axon/libaxon_pjrt.so")
        if hook is not None:
            mod.set_axon_ntff_profile_hook(hook)
    except Exception:
        pass


def _emit(nc, tc, dr, NK, debug=False):
    """Emit the per-core Tile program (projections interleaved with attention)."""
    from contextlib import ExitStack

    with ExitStack() as ctx:
        consts = ctx.enter_context(tc.tile_pool(name="consts", bufs=1))

        # ---- persistent SBUF tiles ----
        KL = NK * P
        VA = consts.tile([P, NK, H, D + 1], BF16)  # V natural + mask col
        OT = consts.tile([P, NC, TQ], BF16)        # O^T  [C, TQ]
        Wo_sb = consts.tile([P, NC, C], BF16)
        boe_rep = consts.tile([P, C], F32)
        lng_rep = consts.tile([P, C], F32)
        lnb_rep = consts.tile([P, C], F32)
        vecs = consts.tile([P, NC, 3], F32)        # bq | bk | maskf
        maskv = consts.tile([P, NK], BF16)
        eps_t = consts.tile([P, 1], F32)

        nc.vector.memset(eps_t, EPS)
        nc.sync.dma_start(out=vecs[:], in_=dr["vecs"].ap()[:])
        nc.sync.dma_start(out=maskv[:], in_=dr["maskv"].ap()[:])

        with (
            tc.tile_pool(name="pa", bufs=1) as pa,
            tc.tile_pool(name="pb", bufs=2) as pb,
            tc.tile_pool(name="pbd", bufs=2, space="DRAM") as pbd,
            tc.tile_pool(name="psP", bufs=2, space="PSUM") as psP,
            tc.tile_pool(name="psS", bufs=2, space="PSUM") as psS,
            tc.tile_pool(name="psO", bufs=2, space="PSUM") as psO,
        ):
            xT = pa.tile([P, NC, KL], BF16)
            xTq = pa.tile([P, NC, TQ], BF16)
            Wq_sb = pa.tile([P, NC, C], BF16)
            Wk_sb = pa.tile([P, NC, C], BF16)
            Wv_sb = pa.tile([P, NC, C], BF16)
            # per-chunk DMAs, ordered to unblock the V projection first
            for kc in range(NC):
                nc.sync.dma_start(out=xT[:, kc, :], in_=dr["xT"].ap()[:, kc, :])
                nc.sync.dma_start(out=Wv_sb[:, kc, :], in_=dr["Wv"].ap()[:, kc, :])
            for kc in range(NC):
                nc.sync.dma_start(out=Wq_sb[:, kc, :], in_=dr["Wq"].ap()[:, kc, :])
                nc.sync.dma_start(out=xTq[:, kc, :], in_=dr["xTq"].ap()[:, kc, :])
                nc.sync.dma_start(out=Wk_sb[:, kc, :], in_=dr["Wk"].ap()[:, kc, :])
            nc.sync.dma_start(out=Wo_sb[:], in_=dr["Wo"].ap()[:])
            for name, rep in (("boe", boe_rep), ("lng", lng_rep), ("lnb", lnb_rep)):
                a = dr[name].ap()
                bcast = bass.AP(
                    tensor=a.tensor, offset=a.offset, ap=[[0, P], [1, C]]
                )
                nc.gpsimd.dma_start(out=rep[:], in_=bcast)

            # ---- V projection: natural [KL, C], masked rows, + mask col ----
            for tcn in range(NK):
                for nn in range(2):
                    ps = psP.tile([P, TQ], F32, tag="psp")
                    for kc in range(NC):
                        nc.tensor.matmul(
                            ps[:],
                            xT[:, kc, tcn * P : (tcn + 1) * P],
                            Wv_sb[:, kc, nn * TQ : (nn + 1) * TQ],
                            start=(kc == 0),
                            stop=(kc == NC - 1),
                        )
                    nc.vector.tensor_scalar_mul(
                        VA[:, tcn, nn * 8 : (nn + 1) * 8, 0:D],
                        ps[:].rearrange("p (h d) -> p h d", h=8),
                        vecs[:, tcn, 2:3],
                    )
                nc.vector.tensor_copy(
                    out=VA[:, tcn, :, D : D + 1],
                    in_=maskv[:, tcn, None].to_broadcast((P, H, 1)),
                )

            # ---- per head-pair: QT/KT projection, S^T, exp, O^T, normalize ----
            for c in range(NPAIR):
                h0, h1 = 2 * c, 2 * c + 1
                QTc = pb.tile([P, TQ], BF16, tag="qtc")
                KTc = pb.tile([P, KL], BF16, tag="ktc")
                ps = psP.tile([P, TQ], F32, tag="psp")
                for kc in range(NC):
                    nc.tensor.matmul(
                        ps[:],
                        Wq_sb[:, kc, c * P : (c + 1) * P],
                        xTq[:, kc, :],
                        start=(kc == 0),
                        stop=(kc == NC - 1),
                    )
                nc.vector.tensor_scalar_add(QTc[:], ps[:], vecs[:, c, 0:1])
                for ko in range(0, KL, TQ):
                    w = min(TQ, KL - ko)
                    ps = psP.tile([P, TQ], F32, tag="psp")
                    for kc in range(NC):
                        nc.tensor.matmul(
                            ps[:, :w],
                            Wk_sb[:, kc, c * P : (c + 1) * P],
                            xT[:, kc, ko : ko + w],
                            start=(kc == 0),
                            stop=(kc == NC - 1),
                        )
                    nc.vector.tensor_scalar_add(
                        KTc[:, ko : ko + w], ps[:, :w], vecs[:, c, 1:2]
                    )

                etb = 2 if NK <= 6 else 1
                et0 = pb.tile([P, NK, TQ], BF16, tag="et0", bufs=etb)
                et1 = pb.tile([P, NK, TQ], BF16, tag="et1", bufs=etb)
                for jj in range((NK + 1) // 2):
                    nu = min(2, NK - 2 * jj)
                    s0 = psS.tile([P, 2 * TQ], F32, tag="s0", bufs=1)
                    s1 = psS.tile([P, 2 * TQ], F32, tag="s1", bufs=1)
                    for u in range(nu):
                        jc = 2 * jj + u
                        js = slice(jc * P, (jc + 1) * P)
                        nc.tensor.matmul(
                            s0[:, u * TQ : (u + 1) * TQ],
                            KTc[0:D, js],
                            QTc[0:D, :],
                            start=True, stop=True,
                            tile_position=(0, 0),
                        )
                        nc.tensor.matmul(
                            s1[:, u * TQ : (u + 1) * TQ],
                            KTc[D:P, js],
                            QTc[D:P, :],
                            start=True, stop=True,
                            tile_position=(D, 0),
                        )
                    nc.scalar.activation(
                        out=et0[:, 2 * jj : 2 * jj + nu, :],
                        in_=s0[:, : nu * TQ],
                        func=mybir.ActivationFunctionType.Exp, scale=0.125,
                    )
                    nc.scalar.activation(
                        out=et1[:, 2 * jj : 2 * jj + nu, :],
                        in_=s1[:, : nu * TQ],
                        func=mybir.ActivationFunctionType.Exp, scale=0.125,
                    )

                # O^T + denominator: lhsT = [V_h*m | m]  -> psum [65, TQ]
                po0 = psO.tile([P, TQ], F32, tag="po0", bufs=1)
                po1 = psO.tile([P, TQ], F32, tag="po1", bufs=1)
                for jc in range(NK):
                    nc.tensor.matmul(
                        po0[0 : D + 1, :], VA[:, jc, h0, :], et0[:, jc, :],
                        start=(jc == 0), stop=(jc == NK - 1),
                    )
                for jc in range(NK):
                    nc.tensor.matmul(
                        po1[0 : D + 1, :], VA[:, jc, h1, :], et1[:, jc, :],
                        start=(jc == 0), stop=(jc == NK - 1),
                    )

                # d rows live on psum partition 64: copy out, shift to
                # partition 0 (approx-recip ucode is broken at base!=0),
                # reciprocal, bounce through DRAM for the partition-bcast.
                dsb = pb.tile([P, 2 * TQ], F32, tag="dsb", bufs=1)
                dp0 = pb.tile([1, 2 * TQ], F32, tag="dp0", bufs=1)
                rp0 = pb.tile([1, 2 * TQ], F32, tag="rp0", bufs=1)
                nc.vector.tensor_copy(out=dsb[D : D + 1, 0:TQ], in_=po0[D : D + 1, :])
                nc.vector.tensor_copy(out=dsb[D : D + 1, TQ:], in_=po1[D : D + 1, :])
                nc.gpsimd.dma_start(out=dp0[0:1, :], in_=dsb[D : D + 1, :])
                nc.vector.reciprocal_approx_fast(out=rp0[:], in_=dp0[:])
                rrep = pb.tile([D, 2 * TQ], F32, tag="rrep")
                rdram = pbd.tile([1, 2 * TQ], F32, tag="rdram")
                nc.sync.dma_start(out=rdram[:], in_=rp0[0:1, :])
                for u in range(2):
                    src = rdram[0:1, u * TQ : (u + 1) * TQ]
                    bcast = bass.AP(
                        tensor=src.tensor, offset=src.offset, ap=[[0, D]] + src.ap[1:]
                    )
                    nc.gpsimd.dma_start(
                        out=rrep[:, u * TQ : (u + 1) * TQ], in_=bcast
                    )
                # normalize: even head straight into OT, odd staged + DMA shift
                odd = pb.tile([D, TQ], BF16, tag="odd")
                nc.vector.tensor_tensor(
                    OT[0:D, c, :], po0[0:D, :], rrep[:, 0:TQ], mybir.AluOpType.mult
                )
                nc.vector.tensor_tensor(
                    odd[:], po1[0:D, :], rrep[:, TQ:], mybir.AluOpType.mult
                )
                nc.gpsimd.dma_start(out=OT[D:P, c, :], in_=odd[:])
                if debug and c == 0:
                    nc.sync.dma_start(out=dr["dqt"].ap()[:], in_=QTc[:])
                    nc.sync.dma_start(out=dr["dkt"].ap()[:], in_=KTc[:])
                    nc.sync.dma_start(out=dr["ddsb"].ap()[:], in_=dsb[:])
                    nc.sync.dma_start(out=dr["drsb"].ap()[0:1, :], in_=rp0[0:1, :])
            if debug:
                nc.sync.dma_start(out=dr["dva"].ap()[:], in_=VA[:])
                nc.sync.dma_start(out=dr["dot"].ap()[:], in_=OT[:])

            # ---- output projection + LN, sharing the psP pool (no barrier) ----
            Y = pb.tile([P, NQ, C], F32, tag="ysb", bufs=1)
            for qc in range(NQ):
                qs = slice(qc * P, (qc + 1) * P)
                for nn in range(2):
                    ps = psP.tile([P, TQ], F32, tag="psp")
                    for mc in range(NC):
                        nc.tensor.matmul(
                            ps[:],
                            OT[:, mc, qs],
                            Wo_sb[:, mc, nn * TQ : (nn + 1) * TQ],
                            start=(mc == 0),
                            stop=(mc == NC - 1),
                        )
                    nc.vector.tensor_tensor(
                        Y[:, qc, nn * TQ : (nn + 1) * TQ],
                        ps[:],
                        boe_rep[:, nn * TQ : (nn + 1) * TQ],
                        mybir.AluOpType.add,
                    )
                # layernorm over free dim (C)
                fmax = nc.vector.BN_STATS_FMAX
                nsub = (C + fmax - 1) // fmax
                stats = pb.tile([P, nsub, nc.vector.BN_STATS_DIM], F32, tag="stats")
                mv = pb.tile([P, nc.vector.BN_AGGR_DIM], F32, tag="mv")
                yq = Y[:, qc, :].rearrange("p (s d) -> p s d", s=nsub)
                for s in range(nsub):
                    nc.vector.bn_stats(out=stats[:, s, :], in_=yq[:, s, :])
                nc.vector.bn_aggr(out=mv[:], in_=stats[:])
                rstd = pb.tile([P, 1], F32, tag="rstd")
                nmr = pb.tile([P, 1], F32, tag="nmr")
                nc.scalar.activation(
                    out=rstd[:], in_=mv[:, 1:2],
                    func=mybir.ActivationFunctionType.Sqrt,
                    bias=eps_t[:], scale=1.0,
                )
                nc.vector.reciprocal(out=rstd[:], in_=rstd[:])
                nc.vector.tensor_scalar(
                    nmr[:], mv[:, 0:1], rstd[:], -1.0,
                    mybir.AluOpType.mult, mybir.AluOpType.mult,
                )
                nc.scalar.activation(
                    out=Y[:, qc, :], in_=Y[:, qc, :],
                    func=mybir.ActivationFunctionType.Identity,
                    bias=nmr[:], scale=rstd[:],
                )
                nc.vector.tensor_tensor(
                    Y[:, qc, :], Y[:, qc, :], lng_rep[:], mybir.AluOpType.mult
                )
                nc.vector.tensor_tensor(
                    Y[:, qc, :], Y[:, qc, :], lnb_rep[:], mybir.AluOpType.add
                )
                nc.sync.dma_start(out=dr["y"].ap()[qs, :], in_=Y[:, qc, :])


def _build(NK=NT, debug=False):
    nc = bacc.Bacc("TRN2", target_bir_lowering=False, debug=False, num_devices=8)
    dr = {}
    dr["xT"] = nc.dram_tensor("xT", [P, NC, NK * P], BF16, kind="ExternalInput")
    dr["xTq"] = nc.dram_tensor("xTq", [P, NC, TQ], BF16, kind="ExternalInput")
    for w in ("Wq", "Wk", "Wv", "Wo"):
        dr[w] = nc.dram_tensor(w, [P, NC, C], BF16, kind="ExternalInput")
    dr["vecs"] = nc.dram_tensor("vecs", [P, NC, 3], F32, kind="ExternalInput")
    dr["maskv"] = nc.dram_tensor("maskv", [P, NK], BF16, kind="ExternalInput")
    for v in ("boe", "lng", "lnb"):
        dr[v] = nc.dram_tensor(v, [1, C], F32, kind="ExternalInput")
    dr["y"] = nc.dram_tensor("y", [TQ, C], F32, kind="ExternalOutput")
    if debug:
        dr["dqt"] = nc.dram_tensor("dqt", [P, TQ], BF16, kind="ExternalOutput")
        dr["dkt"] = nc.dram_tensor("dkt", [P, NK * P], BF16, kind="ExternalOutput")
        dr["dva"] = nc.dram_tensor(
            "dva", [P, NK, H, D + 1], BF16, kind="ExternalOutput"
        )
        dr["ddsb"] = nc.dram_tensor("ddsb", [P, 2 * TQ], F32, kind="ExternalOutput")
        dr["drsb"] = nc.dram_tensor("drsb", [P, 2 * TQ], F32, kind="ExternalOutput")
        dr["dot"] = nc.dram_tensor("dot", [P, NC, TQ], BF16, kind="ExternalOutput")
    with tile.TileContext(nc) as tc:
        _emit(nc, tc, dr, NK, debug=debug)
    nc.compile()
    return nc


def _chunk(a):
    """[C, N] -> [128, C//128, N] with [p, c, n] = a[128c+p, n]."""
    return np.ascontiguousarray(
        a.reshape(NC, P, -1).transpose(1, 0, 2)
    )


def kernel(**inputs):
    global LAST_RESULTS
    dbg = os.environ.get("KERNEL_DEBUG", "0") == "1"
    f32 = np.float32
    Wq = np.asarray(inputs["Wq"], f32)
    Wk = np.asarray(inputs["Wk"], f32)
    Wv = np.asarray(inputs["Wv"], f32)
    Wo = np.asarray(inputs["Wo"], f32)
    x = np.asarray(inputs["x"], f32)
    mask = np.asarray(inputs["attn_mask"]).reshape(B, T)
    # sort keys so unmasked come first; masked tail chunks are dropped
    perms = [np.argsort(-mask[b], kind="stable") for b in range(B)]
    m1max = max(int(mask[b].sum()) for b in range(B))
    NK = min(NT, max(1, -(-m1max // P)))
    KL = NK * P
    key = ("nc", NK, dbg)
    if key not in _CACHE:
        _CACHE[key] = _build(NK=NK, debug=dbg)
    nc = _CACHE[key]
    bq = np.asarray(inputs["bq"], f32)
    bk = np.asarray(inputs["bk"], f32)
    bv = np.asarray(inputs["bv"], f32)
    bo = np.asarray(inputs["bo"], f32)
    ln_g = np.asarray(inputs["ln_g"], f32)
    ln_b = np.asarray(inputs["ln_b"], f32)

    shared = {
        "Wq": _chunk(Wq).astype(NPBF16),
        "Wk": _chunk(Wk).astype(NPBF16),
        "Wv": _chunk(Wv).astype(NPBF16),
        "Wo": _chunk(Wo).astype(NPBF16),
        "boe": (bv @ Wo + bo).reshape(1, C).astype(f32),
        "lng": ln_g.reshape(1, C).astype(f32),
        "lnb": ln_b.reshape(1, C).astype(f32),
    }
    in_maps = []
    for core in range(8):
        b, half = core // 2, core % 2
        xt = np.ascontiguousarray(x[b].T)  # [C, T]
        pk = perms[b][:KL]
        mfp = mask[b][pk].astype(f32)     # permuted/truncated key mask
        vcol = np.zeros((P, NC), f32)
        vcol[:, :NK] = mfp.reshape(NK, P).T
        vecs = np.stack([bq.reshape(NC, P).T, bk.reshape(NC, P).T, vcol], axis=-1)
        m = dict(shared)
        m["xT"] = _chunk(np.ascontiguousarray(xt[:, pk])).astype(NPBF16)
        m["xTq"] = _chunk(xt[:, half * TQ : (half + 1) * TQ]).astype(NPBF16)
        m["vecs"] = np.ascontiguousarray(vecs, f32)
        m["maskv"] = np.ascontiguousarray(mfp.reshape(NK, P).T.astype(NPBF16))
        in_maps.append(m)

    trace = os.environ.get("KERNEL_TRACE", "0") == "1"
    if trace:
        _ensure_ntff_hook()
    LAST_RESULTS = run_bass_kernel_spmd(
        nc, in_maps, core_ids=list(range(8)), trace=trace
    )
    out = np.empty((B, T, C), f32)
    for core in range(8):
        b, half = core // 2, core % 2
        out[b, half * TQ : (half + 1) * TQ, :] = LAST_RESULTS.results[core]["y"]
    return out
